# revision 23
# baseline (speedup 1.0000x reference)
"""MoE (top-2 of 8 experts, SwiGLU) Trainium2 kernel.

Strategy (expert-parallel over 8 NeuronCores):
  * Host: router GEMM + top-2 + sigmoid gates in numpy (selection verified to
    match the jax fp32 reference on these inputs), then gather each expert's
    tokens into a transposed, capacity-padded buffer xT_e [H, C]. One expert
    per core.
  * Device (SPMD, per core): two phases.
      Phase 1: h = silu(x @ Wg) * (x @ Wu), Wg/Wu SBUF-resident, h spilled
               to a DRAM scratch buffer (layout [I, C]).
      Phase 2: yT = (h @ Wd) * gate, Wd SBUF-resident, gate applied during
               PSUM eviction (out column t scaled by gate[t]).
    Matmuls run in float32r — IEEE fp32 layout with the mantissa rounded to
    11 bits (low 12 bits zero), which streams at full PE rate (1 cycle/row
    for moving dim >= 256) vs 4 cycles/row for fp32. Inputs are pre-rounded
    on the host (round-to-nearest-even bit trick); the h intermediate is
    rounded on-chip for free by giving the DVE multiply an fp32r output.
    Tokens are the moving dimension (512 wide), weights the 128x128
    stationary operand.
  * Host: out[idx_e] += yT_e[:, :n_e].T  (indices within one expert are
    unique, so fancy-index += is safe).
"""

import os
import numpy as np

T, H, I, E, TOPK = 8192, 1024, 2048, 8, 2
NCORES = 8
PB = 128

_compiled = {}
last_results = None  # BassKernelResults of the most recent run (for test harness)


def round_fp32r(a):
    """Round fp32 array to fp32r (11-bit mantissa, RNE), keeping fp32 layout."""
    u = np.ascontiguousarray(a, dtype=np.float32).view(np.uint32)
    r = (u + np.uint32(0x7FF) + ((u >> np.uint32(12)) & np.uint32(1))) \
        & np.uint32(0xFFFFF000)
    return r.view(np.float32)


def _tsegs(C):
    segs = []
    t0 = 0
    while t0 < C:
        tb = 512 if C - t0 >= 512 else (C - t0)
        segs.append((t0, tb))
        t0 += tb
    return segs


def _build(C):
    import concourse.bacc as bacc
    import concourse.mybir as mybir
    import concourse.tile as tile

    fp32 = mybir.dt.float32
    fp32r = mybir.dt.float32r
    AF = mybir.ActivationFunctionType

    KB = H // PB   # 8 contraction blocks over H
    IB = I // PB   # 16 blocks over I
    HB = H // PB   # 8 output blocks over H

    nc = bacc.Bacc("TRN2", target_bir_lowering=False, debug=False,
                   num_devices=NCORES)
    xT = nc.dram_tensor("xT", [H, C], fp32r, kind="ExternalInput").ap()
    gm = nc.dram_tensor("gm", [PB, C], fp32, kind="ExternalInput").ap()
    Wg = nc.dram_tensor("Wg", [H, I], fp32r, kind="ExternalInput").ap()
    Wu = nc.dram_tensor("Wu", [H, I], fp32r, kind="ExternalInput").ap()
    Wd = nc.dram_tensor("Wd", [I, H], fp32r, kind="ExternalInput").ap()
    yT = nc.dram_tensor("yT", [H, C], fp32, kind="ExternalOutput").ap()
    hsp = nc.dram_tensor("hsp", [I, C], fp32r, kind="Internal").ap()

    segs = _tsegs(C)

    QW = 512           # weight-column quarter width
    NQ = I // QW       # 4 quarters
    IPQ = QW // PB     # 4 i-blocks per quarter

    # Split token segments into pair-groups so per-group x tiles fit in SBUF
    # while the quarter loop runs outermost (weights stream exactly once).
    halves = [segs[i:i + 2] for i in range(0, len(segs), 2)]
    N_EARLY = 4 if len(segs) > 1 else 0   # Wd tiles preloaded during phase 1

    from contextlib import ExitStack
    with tile.TileContext(nc) as tc, ExitStack() as _stack:
        wde = _stack.enter_context(tc.tile_pool(name="wde", bufs=1, side="right"))
        # Phase 1: h = silu(x@Wg) * (x@Wu) -> DRAM spill (fp32r)
        with tc.tile_pool(name="w1", bufs=1) as w1, \
             tc.tile_pool(name="xp", bufs=1) as xp, \
             tc.tile_pool(name="ev1", bufs=2) as ev1, \
             tc.tile_pool(name="ps1", bufs=3, space="PSUM") as ps1:
            wg_s = [[None] * NQ for _ in range(KB)]
            wu_s = [[None] * NQ for _ in range(KB)]

            def load_xt(si, t0, tb):
                tiles = []
                for k in range(KB):
                    xtk = xp.tile([PB, tb], fp32r, tag=f"xt{k}_{si}",
                                  name=f"xt{k}_{si}")
                    nc.sync.dma_start(
                        out=xtk[:], in_=xT[k * PB:(k + 1) * PB, t0:t0 + tb])
                    tiles.append(xtk)
                return tiles

            # q0 weight tiles live in their own pool, closed after their last
            # use so the freed SBUF can preload Wd tiles before phase 2.
            w1q0_stack = ExitStack()
            w1q0 = w1q0_stack.enter_context(tc.tile_pool(name="w1q0", bufs=1))

            # Interleave the first x tiles with the q0 gate weights so the
            # first matmul can issue after ~0.5MB of DMA.
            xt_half = []
            t0_0, tb_0 = halves[0][0]
            first_xt = []
            for k in range(KB):
                xtk = xp.tile([PB, tb_0], fp32r, tag=f"xt{k}_0", name=f"xt{k}_0")
                nc.sync.dma_start(
                    out=xtk[:], in_=xT[k * PB:(k + 1) * PB, t0_0:t0_0 + tb_0])
                first_xt.append(xtk)
                wgk = w1q0.tile([PB, QW], fp32r, name=f"wg{k}_0")
                nc.sync.dma_start(out=wgk[:], in_=Wg[k * PB:(k + 1) * PB, 0:QW])
                wg_s[k][0] = wgk
            for k in range(KB):
                wuk = w1q0.tile([PB, QW], fp32r, name=f"wu{k}_0")
                nc.sync.dma_start(out=wuk[:], in_=Wu[k * PB:(k + 1) * PB, 0:QW])
                wu_s[k][0] = wuk
            # Rest of half-0 x tiles, then remaining weight quarters.
            xt_half.append([first_xt] + [load_xt(si, t0, tb)
                                         for si, (t0, tb)
                                         in enumerate(halves[0][1:], start=1)])
            for q in range(1, NQ):
                for k in range(KB):
                    wgk = w1.tile([PB, QW], fp32r, name=f"wg{k}_{q}")
                    nc.sync.dma_start(
                        out=wgk[:], in_=Wg[k * PB:(k + 1) * PB, q * QW:(q + 1) * QW])
                    wg_s[k][q] = wgk
                for k in range(KB):
                    wuk = w1.tile([PB, QW], fp32r, name=f"wu{k}_{q}")
                    nc.sync.dma_start(
                        out=wuk[:], in_=Wu[k * PB:(k + 1) * PB, q * QW:(q + 1) * QW])
                    wu_s[k][q] = wuk
            # Preload the first Wd tiles during phase 1 (disjoint SBUF).
            wd_early = []
            for ib in range(N_EARLY):
                wdk = wde.tile([PB, H], fp32r, name=f"wde{ib}")
                nc.sync.dma_start(out=wdk[:], in_=Wd[ib * PB:(ib + 1) * PB, :])
                wd_early.append(wdk)

            for hi, half in enumerate(halves):
                if hi > 0:
                    xt_half.append([load_xt(si, t0, tb)
                                    for si, (t0, tb) in enumerate(half)])
                for q in range(NQ):
                    if hi == len(halves) - 1 and q == 1:
                        # q0 is dead everywhere: release its SBUF and use it
                        # to preload 8 more Wd tiles during the phase-1 tail.
                        w1q0_stack.close()
                        w2a = _stack.enter_context(
                            tc.tile_pool(name="w2a", bufs=1, side="right"))
                        for ib in range(N_EARLY, min(IB, N_EARLY + 8)):
                            wdk = w2a.tile([PB, H], fp32r, name=f"wda{ib}")
                            nc.sync.dma_start(
                                out=wdk[:], in_=Wd[ib * PB:(ib + 1) * PB, :])
                            wd_early.append(wdk)
                    for si, (t0, tb) in enumerate(half):
                        xt = xt_half[hi][si]
                        for ii in range(IPQ):
                            ib = q * IPQ + ii
                            qc = ii * PB
                            pg = ps1.tile([PB, tb], fp32, tag="pg", name="pg")
                            pu = ps1.tile([PB, tb], fp32, tag="pu", name="pu")
                            for k in range(KB):
                                nc.tensor.matmul(
                                    pg[:],
                                    wg_s[k][q][:, qc:qc + PB],
                                    xt[k][:],
                                    start=(k == 0), stop=(k == KB - 1))
                            for k in range(KB):
                                nc.tensor.matmul(
                                    pu[:],
                                    wu_s[k][q][:, qc:qc + PB],
                                    xt[k][:],
                                    start=(k == 0), stop=(k == KB - 1))
                            sg = ev1.tile([PB, tb], fp32, tag="sg", name="sg")
                            nc.scalar.activation(sg[:], pg[:], AF.Sigmoid)
                            sx = ev1.tile([PB, tb], fp32, tag="sx", name="sx")
                            nc.vector.tensor_mul(sx[:], sg[:], pg[:])
                            hh = ev1.tile([PB, tb], fp32r, tag="hh", name="hh")
                            nc.vector.tensor_mul(hh[:], sx[:], pu[:])
                            nc.gpsimd.dma_start(
                                out=hsp[ib * PB:(ib + 1) * PB, t0:t0 + tb],
                                in_=hh[:])

        # Phase 2: yT = (h @ Wd) * gate.  ib-outer: all 8 output blocks
        # accumulate in 8 PSUM banks so compute starts after wd0+ht0 land.
        with tc.tile_pool(name="w2", bufs=1) as w2, \
             tc.tile_pool(name="hl", bufs=3) as hl, \
             tc.tile_pool(name="ev2", bufs=3) as ev2, \
             tc.tile_pool(name="ps2", bufs=1, space="PSUM") as ps2:
            def load_ht(t0, tb, ib):
                htk = hl.tile([PB, tb], fp32r, tag=f"ht{ib}", name=f"ht{ib}")
                nc.sync.dma_start(
                    out=htk[:], in_=hsp[ib * PB:(ib + 1) * PB, t0:t0 + tb])
                return htk

            # Interleave remaining wd tiles with seg-0 h tiles in need-order.
            wd_s = list(wd_early)
            ht_next = []   # seg0 tiles
            for ib in range(IB):
                if ib >= len(wd_early):
                    wdk = w2.tile([PB, H], fp32r, name=f"wd{ib}")
                    nc.sync.dma_start(out=wdk[:], in_=Wd[ib * PB:(ib + 1) * PB, :])
                    wd_s.append(wdk)
                ht_next.append(load_ht(segs[0][0], segs[0][1], ib))
            gt = w2.tile([PB, C], fp32, name="gt")
            nc.sync.dma_start(out=gt[:], in_=gm[:])
            for si, (t0, tb) in enumerate(segs):
                ht = ht_next
                # queue the next segment's h tiles
                if si + 1 < len(segs):
                    nt0, ntb = segs[si + 1]
                    ht_next = [load_ht(nt0, ntb, ib) for ib in range(IB)]
                py = [ps2.tile([PB, tb], fp32, tag=f"py{hb}", name=f"py{hb}")
                      for hb in range(HB)]
                for ib in range(IB):
                    last = ib == IB - 1
                    for hb in range(HB):
                        nc.tensor.matmul(
                            py[hb][:],
                            wd_s[ib][:, hb * PB:(hb + 1) * PB],
                            ht[ib][:],
                            start=(ib == 0), stop=last)
                        if last:
                            # evict as soon as this output block finishes
                            yt = ev2.tile([PB, tb], fp32, tag="yt", name="yt")
                            nc.vector.tensor_mul(yt[:], py[hb][:],
                                                 gt[:, t0:t0 + tb])
                            nc.gpsimd.dma_start(
                                out=yT[hb * PB:(hb + 1) * PB, t0:t0 + tb],
                                in_=yt[:])
    nc.compile()
    return nc


def _route(x, Wr, br):
    """Replicate the reference's fp32 router bit-compatibly on host."""
    logits = x @ Wr + br                       # fp32 GEMM
    order = np.argsort(-logits, axis=1, kind="stable")  # ties -> lowest index
    topk_idx = order[:, :TOPK]
    topk_vals = np.take_along_axis(logits, topk_idx, axis=1)
    g = 1.0 / (1.0 + np.exp(-topk_vals.astype(np.float32)))
    g = g / (np.sum(g, axis=-1, keepdims=True) + 1e-10)
    return topk_idx, g.astype(np.float32)


def kernel(x, Wr, br, Wg, Wu, Wd):
    global last_results
    from concourse.bass_utils import run_bass_kernel_spmd

    x = np.asarray(x, dtype=np.float32)
    Wr = np.asarray(Wr, dtype=np.float32)
    br = np.asarray(br, dtype=np.float32)
    Wg = np.asarray(Wg, dtype=np.float32)
    Wu = np.asarray(Wu, dtype=np.float32)
    Wd = np.asarray(Wd, dtype=np.float32)

    topk_idx, g = _route(x, Wr, br)

    # Per-expert token lists
    idx_lists = []
    gate_lists = []
    for e in range(E):
        mask = topk_idx == e                    # [T, K]
        tok = np.nonzero(mask.any(axis=1))[0]
        # gate value for expert e per selected token (slot 0 or slot 1)
        gsel = np.where(mask[tok, 0], g[tok, 0], g[tok, 1]).astype(np.float32)
        idx_lists.append(tok.astype(np.int64))
        gate_lists.append(gsel)

    counts = [len(ix) for ix in idx_lists]
    C = max(512, -(-max(counts) // 256) * 256)

    key = C
    if key not in _compiled:
        _compiled[key] = _build(C)
    nc = _compiled[key]

    xTf = round_fp32r(np.ascontiguousarray(x.T))   # [H, T], pre-rounded
    in_maps = []
    for e in range(E):
        n = counts[e]
        xTe = np.zeros((H, C), dtype=np.float32)
        xTe[:, :n] = xTf[:, idx_lists[e]]
        gme = np.zeros((PB, C), dtype=np.float32)
        gme[:, :n] = gate_lists[e][None, :]
        in_maps.append({
            "xT": xTe,
            "gm": gme,
            "Wg": round_fp32r(Wg[e]),
            "Wu": round_fp32r(Wu[e]),
            "Wd": round_fp32r(Wd[e]),
        })

    trace = bool(int(os.environ.get("MOE_TRACE", "0")))
    trace_cores = (list(range(NCORES))
                   if os.environ.get("MOE_TRACE_ALL") else None)
    last_results = run_bass_kernel_spmd(
        nc, in_maps, core_ids=list(range(NCORES)), trace=trace,
        trace_cores=trace_cores)

    out = np.zeros((T, H), dtype=np.float32)
    for e in range(E):
        n = counts[e]
        yTe = last_results.results[e]["yT"]
        out[idx_lists[e]] += yTe[:, :n].T
    return out


# revision 25
# speedup vs baseline: 1.0455x; 1.0455x over previous
"""MoE (top-2 of 8 experts, SwiGLU) Trainium2 kernel.

Strategy (expert-parallel over 8 NeuronCores):
  * Host: router GEMM + top-2 + sigmoid gates in numpy (selection verified to
    match the jax fp32 reference on these inputs), then gather each expert's
    tokens into a transposed, capacity-padded buffer xT_e [H, C]. One expert
    per core.
  * Device (SPMD, per core): two phases.
      Phase 1: h = silu(x @ Wg) * (x @ Wu), Wg/Wu SBUF-resident, h spilled
               to a DRAM scratch buffer (layout [I, C]).
      Phase 2: yT = (h @ Wd) * gate, Wd SBUF-resident, gate applied during
               PSUM eviction (out column t scaled by gate[t]).
    Matmuls run in float32r — IEEE fp32 layout with the mantissa rounded to
    11 bits (low 12 bits zero), which streams at full PE rate (1 cycle/row
    for moving dim >= 256) vs 4 cycles/row for fp32. Inputs are pre-rounded
    on the host (round-to-nearest-even bit trick); the h intermediate is
    rounded on-chip for free by giving the DVE multiply an fp32r output.
    Tokens are the moving dimension (512 wide), weights the 128x128
    stationary operand.
  * Host: out[idx_e] += yT_e[:, :n_e].T  (indices within one expert are
    unique, so fancy-index += is safe).
"""

import os
import numpy as np

T, H, I, E, TOPK = 8192, 1024, 2048, 8, 2
NCORES = 8
PB = 128

_compiled = {}
last_results = None  # BassKernelResults of the most recent run (for test harness)


def round_fp32r(a):
    """Round fp32 array to fp32r (11-bit mantissa, RNE), keeping fp32 layout."""
    u = np.ascontiguousarray(a, dtype=np.float32).view(np.uint32)
    r = (u + np.uint32(0x7FF) + ((u >> np.uint32(12)) & np.uint32(1))) \
        & np.uint32(0xFFFFF000)
    return r.view(np.float32)


def _tsegs(C):
    """Split C into segments of width 256..512 (fp32r full rate needs >=256)."""
    widths = []
    rem = C
    while rem >= 768:
        widths.append(512)
        rem -= 512
    if rem <= 512:
        widths.append(rem)
    else:
        widths.append(rem - 256)
        widths.append(256)
    segs = []
    t0 = 0
    for tb in widths:
        segs.append((t0, tb))
        t0 += tb
    return segs


def _build(C):
    import concourse.bacc as bacc
    import concourse.mybir as mybir
    import concourse.tile as tile

    fp32 = mybir.dt.float32
    fp32r = mybir.dt.float32r
    AF = mybir.ActivationFunctionType

    KB = H // PB   # 8 contraction blocks over H
    IB = I // PB   # 16 blocks over I
    HB = H // PB   # 8 output blocks over H

    nc = bacc.Bacc("TRN2", target_bir_lowering=False, debug=False,
                   num_devices=NCORES)
    xT = nc.dram_tensor("xT", [H, C], fp32r, kind="ExternalInput").ap()
    gm = nc.dram_tensor("gm", [PB, C], fp32, kind="ExternalInput").ap()
    Wg = nc.dram_tensor("Wg", [H, I], fp32r, kind="ExternalInput").ap()
    Wu = nc.dram_tensor("Wu", [H, I], fp32r, kind="ExternalInput").ap()
    Wd = nc.dram_tensor("Wd", [I, H], fp32r, kind="ExternalInput").ap()
    yT = nc.dram_tensor("yT", [H, C], fp32, kind="ExternalOutput").ap()
    hsp = nc.dram_tensor("hsp", [I, C], fp32r, kind="Internal").ap()

    segs = _tsegs(C)

    QW = 512           # weight-column quarter width
    NQ = I // QW       # 4 quarters
    IPQ = QW // PB     # 4 i-blocks per quarter

    # Split token segments into pair-groups so per-group x tiles fit in SBUF
    # while the quarter loop runs outermost (weights stream exactly once).
    halves = [segs[i:i + 2] for i in range(0, len(segs), 2)]
    N_EARLY = 4 if len(segs) > 1 else 0   # Wd tiles preloaded during phase 1

    from contextlib import ExitStack
    with tile.TileContext(nc) as tc, ExitStack() as _stack:
        wde = _stack.enter_context(tc.tile_pool(name="wde", bufs=1, side="right"))
        # Phase 1: h = silu(x@Wg) * (x@Wu) -> DRAM spill (fp32r)
        with tc.tile_pool(name="w1", bufs=1) as w1, \
             tc.tile_pool(name="xp", bufs=1) as xp, \
             tc.tile_pool(name="ev1", bufs=2) as ev1, \
             tc.tile_pool(name="ps1", bufs=3, space="PSUM") as ps1:
            wg_s = [[None] * NQ for _ in range(KB)]
            wu_s = [[None] * NQ for _ in range(KB)]

            def load_xt(si, t0, tb):
                tiles = []
                for k in range(KB):
                    xtk = xp.tile([PB, tb], fp32r, tag=f"xt{k}_{si}",
                                  name=f"xt{k}_{si}")
                    nc.sync.dma_start(
                        out=xtk[:], in_=xT[k * PB:(k + 1) * PB, t0:t0 + tb])
                    tiles.append(xtk)
                return tiles

            # q0 weight tiles live in their own pool, closed after their last
            # use so the freed SBUF can preload Wd tiles before phase 2.
            w1q0_stack = ExitStack()
            w1q0 = w1q0_stack.enter_context(tc.tile_pool(name="w1q0", bufs=1))

            # Interleave the first x tiles with the q0 gate weights so the
            # first matmul can issue after ~0.5MB of DMA.
            xt_half = []
            t0_0, tb_0 = halves[0][0]
            first_xt = []
            for k in range(KB):
                xtk = xp.tile([PB, tb_0], fp32r, tag=f"xt{k}_0", name=f"xt{k}_0")
                nc.sync.dma_start(
                    out=xtk[:], in_=xT[k * PB:(k + 1) * PB, t0_0:t0_0 + tb_0])
                first_xt.append(xtk)
                wgk = w1q0.tile([PB, QW], fp32r, name=f"wg{k}_0")
                nc.sync.dma_start(out=wgk[:], in_=Wg[k * PB:(k + 1) * PB, 0:QW])
                wg_s[k][0] = wgk
            for k in range(KB):
                wuk = w1q0.tile([PB, QW], fp32r, name=f"wu{k}_0")
                nc.sync.dma_start(out=wuk[:], in_=Wu[k * PB:(k + 1) * PB, 0:QW])
                wu_s[k][0] = wuk
            # Rest of half-0 x tiles, then remaining weight quarters.
            xt_half.append([first_xt] + [load_xt(si, t0, tb)
                                         for si, (t0, tb)
                                         in enumerate(halves[0][1:], start=1)])
            for q in range(1, NQ):
                for k in range(KB):
                    wgk = w1.tile([PB, QW], fp32r, name=f"wg{k}_{q}")
                    nc.sync.dma_start(
                        out=wgk[:], in_=Wg[k * PB:(k + 1) * PB, q * QW:(q + 1) * QW])
                    wg_s[k][q] = wgk
                for k in range(KB):
                    wuk = w1.tile([PB, QW], fp32r, name=f"wu{k}_{q}")
                    nc.sync.dma_start(
                        out=wuk[:], in_=Wu[k * PB:(k + 1) * PB, q * QW:(q + 1) * QW])
                    wu_s[k][q] = wuk
            # Preload the first Wd tiles during phase 1 (disjoint SBUF).
            wd_early = []
            for ib in range(N_EARLY):
                wdk = wde.tile([PB, H], fp32r, name=f"wde{ib}")
                nc.sync.dma_start(out=wdk[:], in_=Wd[ib * PB:(ib + 1) * PB, :])
                wd_early.append(wdk)

            for hi, half in enumerate(halves):
                if hi > 0:
                    xt_half.append([load_xt(si, t0, tb)
                                    for si, (t0, tb) in enumerate(half)])
                for q in range(NQ):
                    if hi == len(halves) - 1 and q == 1:
                        # q0 is dead everywhere: release its SBUF and use it
                        # to preload 8 more Wd tiles during the phase-1 tail.
                        w1q0_stack.close()
                        w2a = _stack.enter_context(
                            tc.tile_pool(name="w2a", bufs=1, side="right"))
                        for ib in range(N_EARLY, min(IB, N_EARLY + 8)):
                            wdk = w2a.tile([PB, H], fp32r, name=f"wda{ib}")
                            nc.sync.dma_start(
                                out=wdk[:], in_=Wd[ib * PB:(ib + 1) * PB, :])
                            wd_early.append(wdk)
                    for si, (t0, tb) in enumerate(half):
                        xt = xt_half[hi][si]
                        for ii in range(IPQ):
                            ib = q * IPQ + ii
                            qc = ii * PB
                            pg = ps1.tile([PB, tb], fp32, tag="pg", name="pg")
                            pu = ps1.tile([PB, tb], fp32, tag="pu", name="pu")
                            for k in range(KB):
                                nc.tensor.matmul(
                                    pg[:],
                                    wg_s[k][q][:, qc:qc + PB],
                                    xt[k][:],
                                    start=(k == 0), stop=(k == KB - 1))
                            for k in range(KB):
                                nc.tensor.matmul(
                                    pu[:],
                                    wu_s[k][q][:, qc:qc + PB],
                                    xt[k][:],
                                    start=(k == 0), stop=(k == KB - 1))
                            sg = ev1.tile([PB, tb], fp32, tag="sg", name="sg")
                            nc.scalar.activation(sg[:], pg[:], AF.Sigmoid)
                            sx = ev1.tile([PB, tb], fp32, tag="sx", name="sx")
                            nc.vector.tensor_mul(sx[:], sg[:], pg[:])
                            hh = ev1.tile([PB, tb], fp32r, tag="hh", name="hh")
                            nc.vector.tensor_mul(hh[:], sx[:], pu[:])
                            nc.gpsimd.dma_start(
                                out=hsp[ib * PB:(ib + 1) * PB, t0:t0 + tb],
                                in_=hh[:])

        # Phase 2: yT = (h @ Wd) * gate.  ib-outer: all 8 output blocks
        # accumulate in 8 PSUM banks so compute starts after wd0+ht0 land.
        with tc.tile_pool(name="w2", bufs=1) as w2, \
             tc.tile_pool(name="hl", bufs=3) as hl, \
             tc.tile_pool(name="ev2", bufs=3) as ev2, \
             tc.tile_pool(name="ps2", bufs=1, space="PSUM") as ps2:
            def load_ht(t0, tb, ib):
                htk = hl.tile([PB, tb], fp32r, tag=f"ht{ib}", name=f"ht{ib}")
                nc.sync.dma_start(
                    out=htk[:], in_=hsp[ib * PB:(ib + 1) * PB, t0:t0 + tb])
                return htk

            # Interleave remaining wd tiles with seg-0 h tiles in need-order.
            wd_s = list(wd_early)
            ht_next = []   # seg0 tiles
            for ib in range(IB):
                if ib >= len(wd_early):
                    wdk = w2.tile([PB, H], fp32r, name=f"wd{ib}")
                    nc.sync.dma_start(out=wdk[:], in_=Wd[ib * PB:(ib + 1) * PB, :])
                    wd_s.append(wdk)
                ht_next.append(load_ht(segs[0][0], segs[0][1], ib))
            gt = w2.tile([PB, C], fp32, name="gt")
            nc.sync.dma_start(out=gt[:], in_=gm[:])
            for si, (t0, tb) in enumerate(segs):
                ht = ht_next
                # queue the next segment's h tiles
                if si + 1 < len(segs):
                    nt0, ntb = segs[si + 1]
                    ht_next = [load_ht(nt0, ntb, ib) for ib in range(IB)]
                py = [ps2.tile([PB, tb], fp32, tag=f"py{hb}", name=f"py{hb}")
                      for hb in range(HB)]
                for ib in range(IB):
                    last = ib == IB - 1
                    for hb in range(HB):
                        nc.tensor.matmul(
                            py[hb][:],
                            wd_s[ib][:, hb * PB:(hb + 1) * PB],
                            ht[ib][:],
                            start=(ib == 0), stop=last)
                        if last:
                            # evict as soon as this output block finishes
                            yt = ev2.tile([PB, tb], fp32, tag="yt", name="yt")
                            nc.vector.tensor_mul(yt[:], py[hb][:],
                                                 gt[:, t0:t0 + tb])
                            nc.gpsimd.dma_start(
                                out=yT[hb * PB:(hb + 1) * PB, t0:t0 + tb],
                                in_=yt[:])
    nc.compile()
    return nc


def _route(x, Wr, br):
    """Replicate the reference's fp32 router bit-compatibly on host."""
    logits = x @ Wr + br                       # fp32 GEMM
    order = np.argsort(-logits, axis=1, kind="stable")  # ties -> lowest index
    topk_idx = order[:, :TOPK]
    topk_vals = np.take_along_axis(logits, topk_idx, axis=1)
    g = 1.0 / (1.0 + np.exp(-topk_vals.astype(np.float32)))
    g = g / (np.sum(g, axis=-1, keepdims=True) + 1e-10)
    return topk_idx, g.astype(np.float32)


def kernel(x, Wr, br, Wg, Wu, Wd):
    global last_results
    from concourse.bass_utils import run_bass_kernel_spmd

    x = np.asarray(x, dtype=np.float32)
    Wr = np.asarray(Wr, dtype=np.float32)
    br = np.asarray(br, dtype=np.float32)
    Wg = np.asarray(Wg, dtype=np.float32)
    Wu = np.asarray(Wu, dtype=np.float32)
    Wd = np.asarray(Wd, dtype=np.float32)

    topk_idx, g = _route(x, Wr, br)

    # Per-expert token lists
    idx_lists = []
    gate_lists = []
    for e in range(E):
        mask = topk_idx == e                    # [T, K]
        tok = np.nonzero(mask.any(axis=1))[0]
        # gate value for expert e per selected token (slot 0 or slot 1)
        gsel = np.where(mask[tok, 0], g[tok, 0], g[tok, 1]).astype(np.float32)
        idx_lists.append(tok.astype(np.int64))
        gate_lists.append(gsel)

    counts = [len(ix) for ix in idx_lists]
    C = max(512, max(counts))

    key = C
    if key not in _compiled:
        _compiled[key] = _build(C)
    nc = _compiled[key]

    xTf = round_fp32r(np.ascontiguousarray(x.T))   # [H, T], pre-rounded
    in_maps = []
    for e in range(E):
        n = counts[e]
        xTe = np.zeros((H, C), dtype=np.float32)
        xTe[:, :n] = xTf[:, idx_lists[e]]
        gme = np.zeros((PB, C), dtype=np.float32)
        gme[:, :n] = gate_lists[e][None, :]
        in_maps.append({
            "xT": xTe,
            "gm": gme,
            "Wg": round_fp32r(Wg[e]),
            "Wu": round_fp32r(Wu[e]),
            "Wd": round_fp32r(Wd[e]),
        })

    trace = bool(int(os.environ.get("MOE_TRACE", "0")))
    trace_cores = (list(range(NCORES))
                   if os.environ.get("MOE_TRACE_ALL") else None)
    last_results = run_bass_kernel_spmd(
        nc, in_maps, core_ids=list(range(NCORES)), trace=trace,
        trace_cores=trace_cores)

    out = np.zeros((T, H), dtype=np.float32)
    for e in range(E):
        n = counts[e]
        yTe = last_results.results[e]["yT"]
        out[idx_lists[e]] += yTe[:, :n].T
    return out


# revision 31
# speedup vs baseline: 1.0718x; 1.0251x over previous
"""MoE (top-2 of 8 experts, SwiGLU) Trainium2 kernel.

Strategy (expert-parallel over 8 NeuronCores):
  * Host: router GEMM + top-2 + sigmoid gates in numpy (selection verified to
    match the jax fp32 reference on these inputs), then gather each expert's
    tokens into a transposed, capacity-padded buffer xT_e [H, C]. One expert
    per core.
  * Device (SPMD, per core): two phases.
      Phase 1: h = silu(x @ Wg) * (x @ Wu), Wg/Wu SBUF-resident, h spilled
               to a DRAM scratch buffer (layout [I, C]).
      Phase 2: yT = (h @ Wd) * gate, Wd SBUF-resident, gate applied during
               PSUM eviction (out column t scaled by gate[t]).
    Matmuls run in float32r — IEEE fp32 layout with the mantissa rounded to
    11 bits (low 12 bits zero), which streams at full PE rate (1 cycle/row
    for moving dim >= 256) vs 4 cycles/row for fp32. Inputs are pre-rounded
    on the host (round-to-nearest-even bit trick); the h intermediate is
    rounded on-chip for free by giving the DVE multiply an fp32r output.
    Tokens are the moving dimension (512 wide), weights the 128x128
    stationary operand.
  * Host: out[idx_e] += yT_e[:, :n_e].T  (indices within one expert are
    unique, so fancy-index += is safe).
"""

import os
import numpy as np

T, H, I, E, TOPK = 8192, 1024, 2048, 8, 2
NCORES = 8
PB = 128

_compiled = {}
last_results = None  # BassKernelResults of the most recent run (for test harness)


def round_fp32r(a):
    """Round fp32 array to fp32r (11-bit mantissa, RNE), keeping fp32 layout."""
    u = np.ascontiguousarray(a, dtype=np.float32).view(np.uint32)
    r = (u + np.uint32(0x7FF) + ((u >> np.uint32(12)) & np.uint32(1))) \
        & np.uint32(0xFFFFF000)
    return r.view(np.float32)


def _tsegs(C):
    """Split C into segments of width 256..512 (fp32r full rate needs >=256)."""
    widths = []
    rem = C
    while rem >= 768:
        widths.append(512)
        rem -= 512
    if rem <= 512:
        widths.append(rem)
    else:
        widths.append(rem - 256)
        widths.append(256)
    segs = []
    t0 = 0
    for tb in widths:
        segs.append((t0, tb))
        t0 += tb
    return segs


def _build(C):
    import concourse.bacc as bacc
    import concourse.mybir as mybir
    import concourse.tile as tile

    fp32 = mybir.dt.float32
    fp32r = mybir.dt.float32r
    AF = mybir.ActivationFunctionType

    KB = H // PB   # 8 contraction blocks over H
    IB = I // PB   # 16 blocks over I
    HB = H // PB   # 8 output blocks over H

    nc = bacc.Bacc("TRN2", target_bir_lowering=False, debug=False,
                   num_devices=NCORES)
    xT = nc.dram_tensor("xT", [H, C], fp32r, kind="ExternalInput").ap()
    gm = nc.dram_tensor("gm", [PB, C], fp32, kind="ExternalInput").ap()
    Wg = nc.dram_tensor("Wg", [H, I], fp32r, kind="ExternalInput").ap()
    Wu = nc.dram_tensor("Wu", [H, I], fp32r, kind="ExternalInput").ap()
    Wd = nc.dram_tensor("Wd", [I, H], fp32r, kind="ExternalInput").ap()
    yT = nc.dram_tensor("yT", [H, C], fp32, kind="ExternalOutput").ap()
    hsp = nc.dram_tensor("hsp", [I, C], fp32r, kind="Internal").ap()

    segs = _tsegs(C)

    QW = 512           # weight-column quarter width
    NQ = I // QW       # 4 quarters
    IPQ = QW // PB     # 4 i-blocks per quarter

    # Split token segments into pair-groups so per-group x tiles fit in SBUF
    # while the quarter loop runs outermost (weights stream exactly once).
    halves = [segs[i:i + 2] for i in range(0, len(segs), 2)]
    N_EARLY = 4 if len(segs) > 1 else 0   # Wd tiles preloaded during phase 1

    from contextlib import ExitStack
    with tile.TileContext(nc) as tc, ExitStack() as _stack:
        wde = _stack.enter_context(tc.tile_pool(name="wde", bufs=1, side="right"))
        # Phase 1: h = silu(x@Wg) * (x@Wu) -> DRAM spill (fp32r)
        with tc.tile_pool(name="w1", bufs=1) as w1, \
             tc.tile_pool(name="xp", bufs=1) as xp, \
             tc.tile_pool(name="ev1", bufs=2) as ev1, \
             tc.tile_pool(name="ps1", bufs=3, space="PSUM") as ps1:
            wg_s = [[None] * NQ for _ in range(KB)]
            wu_s = [[None] * NQ for _ in range(KB)]

            def load_xt(si, t0, tb):
                tiles = []
                for k in range(KB):
                    xtk = xp.tile([PB, tb], fp32r, tag=f"xt{k}_{si}",
                                  name=f"xt{k}_{si}")
                    nc.sync.dma_start(
                        out=xtk[:], in_=xT[k * PB:(k + 1) * PB, t0:t0 + tb])
                    tiles.append(xtk)
                return tiles

            # q0 weight tiles live in their own pool, closed after their last
            # use so the freed SBUF can preload Wd tiles before phase 2.
            w1q0_stack = ExitStack()
            w1q0 = w1q0_stack.enter_context(tc.tile_pool(name="w1q0", bufs=1))

            # Interleave the first x tiles with the q0 gate weights so the
            # first matmul can issue after ~0.5MB of DMA.
            xt_half = []
            t0_0, tb_0 = halves[0][0]
            first_xt = []
            for k in range(KB):
                xtk = xp.tile([PB, tb_0], fp32r, tag=f"xt{k}_0", name=f"xt{k}_0")
                nc.sync.dma_start(
                    out=xtk[:], in_=xT[k * PB:(k + 1) * PB, t0_0:t0_0 + tb_0])
                first_xt.append(xtk)
                wgk = w1q0.tile([PB, QW], fp32r, name=f"wg{k}_0")
                nc.sync.dma_start(out=wgk[:], in_=Wg[k * PB:(k + 1) * PB, 0:QW])
                wg_s[k][0] = wgk
            for k in range(KB):
                wuk = w1q0.tile([PB, QW], fp32r, name=f"wu{k}_0")
                nc.sync.dma_start(out=wuk[:], in_=Wu[k * PB:(k + 1) * PB, 0:QW])
                wu_s[k][0] = wuk
            # Rest of half-0 x tiles, then remaining weight quarters.
            xt_half.append([first_xt] + [load_xt(si, t0, tb)
                                         for si, (t0, tb)
                                         in enumerate(halves[0][1:], start=1)])
            for q in range(1, NQ):
                for k in range(KB):
                    wgk = w1.tile([PB, QW], fp32r, name=f"wg{k}_{q}")
                    nc.sync.dma_start(
                        out=wgk[:], in_=Wg[k * PB:(k + 1) * PB, q * QW:(q + 1) * QW])
                    wg_s[k][q] = wgk
                for k in range(KB):
                    wuk = w1.tile([PB, QW], fp32r, name=f"wu{k}_{q}")
                    nc.sync.dma_start(
                        out=wuk[:], in_=Wu[k * PB:(k + 1) * PB, q * QW:(q + 1) * QW])
                    wu_s[k][q] = wuk
            # Preload the first Wd tiles during phase 1 (disjoint SBUF).
            wd_early = []
            for ib in range(N_EARLY):
                wdk = wde.tile([PB, H], fp32r, name=f"wde{ib}")
                nc.sync.dma_start(out=wdk[:], in_=Wd[ib * PB:(ib + 1) * PB, :])
                wd_early.append(wdk)

            for hi, half in enumerate(halves):
                if hi > 0:
                    xt_half.append([load_xt(si, t0, tb)
                                    for si, (t0, tb) in enumerate(half)])
                for q in range(NQ):
                    if hi == len(halves) - 1 and q == 1:
                        # q0 is dead everywhere: release its SBUF and use it
                        # to preload 8 more Wd tiles during the phase-1 tail.
                        w1q0_stack.close()
                        w2a = _stack.enter_context(
                            tc.tile_pool(name="w2a", bufs=1, side="right"))
                        for ib in range(N_EARLY, min(IB, N_EARLY + 8)):
                            wdk = w2a.tile([PB, H], fp32r, name=f"wda{ib}")
                            nc.sync.dma_start(
                                out=wdk[:], in_=Wd[ib * PB:(ib + 1) * PB, :])
                            wd_early.append(wdk)
                    for si, (t0, tb) in enumerate(half):
                        xt = xt_half[hi][si]
                        for ii in range(IPQ):
                            ib = q * IPQ + ii
                            qc = ii * PB
                            pg = ps1.tile([PB, tb], fp32, tag="pg", name="pg")
                            pu = ps1.tile([PB, tb], fp32, tag="pu", name="pu")
                            for k in range(KB):
                                nc.tensor.matmul(
                                    pg[:],
                                    wg_s[k][q][:, qc:qc + PB],
                                    xt[k][:],
                                    start=(k == 0), stop=(k == KB - 1))
                            for k in range(KB):
                                nc.tensor.matmul(
                                    pu[:],
                                    wu_s[k][q][:, qc:qc + PB],
                                    xt[k][:],
                                    start=(k == 0), stop=(k == KB - 1))
                            sg = ev1.tile([PB, tb], fp32, tag="sg", name="sg")
                            nc.scalar.activation(sg[:], pg[:], AF.Sigmoid)
                            sx = ev1.tile([PB, tb], fp32, tag="sx", name="sx")
                            nc.vector.tensor_mul(sx[:], sg[:], pg[:])
                            hh = ev1.tile([PB, tb], fp32r, tag="hh", name="hh")
                            nc.vector.tensor_mul(hh[:], sx[:], pu[:])
                            nc.gpsimd.dma_start(
                                out=hsp[ib * PB:(ib + 1) * PB, t0:t0 + tb],
                                in_=hh[:])

        # Phase 2: yT = (h @ Wd) * gate.  ib-outer: all 8 output blocks
        # accumulate in 8 PSUM banks so compute starts after wd0+ht0 land.
        with tc.tile_pool(name="w2", bufs=1) as w2, \
             tc.tile_pool(name="hl", bufs=3) as hl, \
             tc.tile_pool(name="ev2", bufs=3) as ev2, \
             tc.tile_pool(name="ps2", bufs=1, space="PSUM") as ps2:
            def load_ht(t0, tb, ib):
                htk = hl.tile([PB, tb], fp32r, tag=f"ht{ib}", name=f"ht{ib}")
                nc.sync.dma_start(
                    out=htk[:], in_=hsp[ib * PB:(ib + 1) * PB, t0:t0 + tb])
                return htk

            # Interleave remaining wd tiles with seg-0 h tiles in need-order.
            wd_s = list(wd_early)
            ht_next = []   # seg0 tiles
            for ib in range(IB):
                if ib >= len(wd_early):
                    wdk = w2.tile([PB, H], fp32r, name=f"wd{ib}")
                    nc.sync.dma_start(out=wdk[:], in_=Wd[ib * PB:(ib + 1) * PB, :])
                    wd_s.append(wdk)
                ht_next.append(load_ht(segs[0][0], segs[0][1], ib))
            gt = w2.tile([PB, C], fp32, name="gt")
            nc.sync.dma_start(out=gt[:], in_=gm[:])
            for si, (t0, tb) in enumerate(segs):
                ht = ht_next
                # queue the next segment's h tiles
                if si + 1 < len(segs):
                    nt0, ntb = segs[si + 1]
                    ht_next = [load_ht(nt0, ntb, ib) for ib in range(IB)]
                py = [ps2.tile([PB, tb], fp32, tag=f"py{hb}", name=f"py{hb}")
                      for hb in range(HB)]
                for ib in range(IB):
                    last = ib == IB - 1
                    for hb in range(HB):
                        nc.tensor.matmul(
                            py[hb][:],
                            wd_s[ib][:, hb * PB:(hb + 1) * PB],
                            ht[ib][:],
                            start=(ib == 0), stop=last)
                        if last:
                            # evict as soon as this output block finishes
                            yt = ev2.tile([PB, tb], fp32, tag="yt", name="yt")
                            nc.vector.tensor_mul(yt[:], py[hb][:],
                                                 gt[:, t0:t0 + tb])
                            nc.gpsimd.dma_start(
                                out=yT[hb * PB:(hb + 1) * PB, t0:t0 + tb],
                                in_=yt[:])
    nc.compile()
    return nc


def _route(x, Wr, br):
    """Replicate the reference's fp32 router bit-compatibly on host."""
    logits = x @ Wr + br                       # fp32 GEMM
    order = np.argsort(-logits, axis=1, kind="stable")  # ties -> lowest index
    topk_idx = order[:, :TOPK]
    topk_vals = np.take_along_axis(logits, topk_idx, axis=1)
    g = 1.0 / (1.0 + np.exp(-topk_vals.astype(np.float32)))
    g = g / (np.sum(g, axis=-1, keepdims=True) + 1e-10)
    return topk_idx, g.astype(np.float32)


def kernel(x, Wr, br, Wg, Wu, Wd):
    global last_results
    from concourse.bass_utils import run_bass_kernel_spmd

    x = np.asarray(x, dtype=np.float32)
    Wr = np.asarray(Wr, dtype=np.float32)
    br = np.asarray(br, dtype=np.float32)
    Wg = np.asarray(Wg, dtype=np.float32)
    Wu = np.asarray(Wu, dtype=np.float32)
    Wd = np.asarray(Wd, dtype=np.float32)

    topk_idx, g = _route(x, Wr, br)

    # Per-expert token lists
    idx_lists = []
    gate_lists = []
    for e in range(E):
        mask = topk_idx == e                    # [T, K]
        tok = np.nonzero(mask.any(axis=1))[0]
        # gate value for expert e per selected token (slot 0 or slot 1)
        gsel = np.where(mask[tok, 0], g[tok, 0], g[tok, 1]).astype(np.float32)
        idx_lists.append(tok.astype(np.int64))
        gate_lists.append(gsel)

    counts = [len(ix) for ix in idx_lists]
    C = max(512, max(counts))

    key = C
    if key not in _compiled:
        _compiled[key] = _build(C)
    nc = _compiled[key]

    xTf = round_fp32r(np.ascontiguousarray(x.T))   # [H, T], pre-rounded
    in_maps = []
    for e in range(E):
        n = counts[e]
        xTe = np.zeros((H, C), dtype=np.float32)
        xTe[:, :n] = xTf[:, idx_lists[e]]
        gme = np.zeros((PB, C), dtype=np.float32)
        gme[:, :n] = gate_lists[e][None, :]
        in_maps.append({
            "xT": xTe,
            "gm": gme,
            "Wg": round_fp32r(Wg[e]),
            "Wu": round_fp32r(Wu[e]),
            "Wd": round_fp32r(Wd[e]),
        })

    trace = bool(int(os.environ.get("MOE_TRACE", "0")))
    trace_cores = (list(range(NCORES))
                   if os.environ.get("MOE_TRACE_ALL") else None)
    last_results = run_bass_kernel_spmd(
        nc, in_maps, core_ids=list(range(NCORES)), trace=trace,
        trace_cores=trace_cores)

    out = np.zeros((T, H), dtype=np.float32)
    for e in range(E):
        n = counts[e]
        yTe = last_results.results[e]["yT"]
        out[idx_lists[e]] += yTe[:, :n].T
    return out


# revision 33
# speedup vs baseline: 1.1140x; 1.0394x over previous
"""MoE (top-2 of 8 experts, SwiGLU) Trainium2 kernel.

Strategy (expert-parallel over 8 NeuronCores):
  * Host: router GEMM + top-2 + sigmoid gates in numpy (selection verified to
    match the jax fp32 reference on these inputs), then gather each expert's
    tokens into a transposed, capacity-padded buffer xT_e [H, C]. One expert
    per core.
  * Device (SPMD, per core): two phases.
      Phase 1: h = silu(x @ Wg) * (x @ Wu), Wg/Wu SBUF-resident, h spilled
               to a DRAM scratch buffer (layout [I, C]).
      Phase 2: yT = (h @ Wd) * gate, Wd SBUF-resident, gate applied during
               PSUM eviction (out column t scaled by gate[t]).
    Matmuls run in float32r — IEEE fp32 layout with the mantissa rounded to
    11 bits (low 12 bits zero), which streams at full PE rate (1 cycle/row
    for moving dim >= 256) vs 4 cycles/row for fp32. Inputs are pre-rounded
    on the host (round-to-nearest-even bit trick); the h intermediate is
    rounded on-chip for free by giving the DVE multiply an fp32r output.
    Tokens are the moving dimension (512 wide), weights the 128x128
    stationary operand.
  * Host: out[idx_e] += yT_e[:, :n_e].T  (indices within one expert are
    unique, so fancy-index += is safe).
"""

import os
import numpy as np

T, H, I, E, TOPK = 8192, 1024, 2048, 8, 2
NCORES = 8
PB = 128

_compiled = {}
last_results = None  # BassKernelResults of the most recent run (for test harness)


def round_fp32r(a):
    """Round fp32 array to fp32r (11-bit mantissa, RNE), keeping fp32 layout."""
    u = np.ascontiguousarray(a, dtype=np.float32).view(np.uint32)
    r = (u + np.uint32(0x7FF) + ((u >> np.uint32(12)) & np.uint32(1))) \
        & np.uint32(0xFFFFF000)
    return r.view(np.float32)


def _tsegs(C):
    """Split C into segments of width 256..512 (fp32r full rate needs >=256)."""
    widths = []
    rem = C
    while rem >= 768:
        widths.append(512)
        rem -= 512
    if rem <= 512:
        widths.append(rem)
    else:
        widths.append(rem - 256)
        widths.append(256)
    segs = []
    t0 = 0
    for tb in widths:
        segs.append((t0, tb))
        t0 += tb
    return segs


def _build(C):
    import concourse.bacc as bacc
    import concourse.mybir as mybir
    import concourse.tile as tile

    fp32 = mybir.dt.float32
    fp32r = mybir.dt.float32r
    AF = mybir.ActivationFunctionType

    KB = H // PB   # 8 contraction blocks over H
    IB = I // PB   # 16 blocks over I
    HB = H // PB   # 8 output blocks over H

    nc = bacc.Bacc("TRN2", target_bir_lowering=False, debug=False,
                   num_devices=NCORES)
    xT = nc.dram_tensor("xT", [H, C], fp32r, kind="ExternalInput").ap()
    gm = nc.dram_tensor("gm", [PB, C], fp32, kind="ExternalInput").ap()
    Wg = nc.dram_tensor("Wg", [H, I], fp32r, kind="ExternalInput").ap()
    Wu = nc.dram_tensor("Wu", [H, I], fp32r, kind="ExternalInput").ap()
    Wd = nc.dram_tensor("Wd", [I, H], fp32r, kind="ExternalInput").ap()
    yT = nc.dram_tensor("yT", [H, C], fp32, kind="ExternalOutput").ap()
    hsp = nc.dram_tensor("hsp", [I, C], fp32r, kind="Internal").ap()

    segs = _tsegs(C)

    QW = 512           # weight-column quarter width
    NQ = I // QW       # 4 quarters
    IPQ = QW // PB     # 4 i-blocks per quarter

    # Split token segments into pair-groups so per-group x tiles fit in SBUF
    # while the quarter loop runs outermost (weights stream exactly once).
    halves = [segs[i:i + 2] for i in range(0, len(segs), 2)]
    N_EARLY = 4 if len(segs) > 1 else 0   # Wd tiles preloaded during phase 1

    from contextlib import ExitStack
    with tile.TileContext(nc) as tc, ExitStack() as _stack:
        wde = _stack.enter_context(tc.tile_pool(name="wde", bufs=1, side="right"))
        # Phase 1: h = silu(x@Wg) * (x@Wu) -> DRAM spill (fp32r)
        with tc.tile_pool(name="w1", bufs=1) as w1, \
             tc.tile_pool(name="xp", bufs=1) as xp, \
             tc.tile_pool(name="ev1", bufs=2) as ev1, \
             tc.tile_pool(name="ps1", bufs=3, space="PSUM") as ps1:
            wg_s = [[None] * NQ for _ in range(KB)]
            wu_s = [[None] * NQ for _ in range(KB)]

            def load_xt(si, t0, tb):
                tiles = []
                for k in range(KB):
                    xtk = xp.tile([PB, tb], fp32r, tag=f"xt{k}_{si}",
                                  name=f"xt{k}_{si}")
                    nc.sync.dma_start(
                        out=xtk[:], in_=xT[k * PB:(k + 1) * PB, t0:t0 + tb])
                    tiles.append(xtk)
                return tiles

            # q0 weight tiles live in their own pool, closed after their last
            # use so the freed SBUF can preload Wd tiles before phase 2.
            w1q0_stack = ExitStack()
            w1q0 = w1q0_stack.enter_context(tc.tile_pool(name="w1q0", bufs=1))

            # Interleave the first x tiles with the q0 gate weights so the
            # first matmul can issue after ~0.5MB of DMA.
            xt_half = []
            t0_0, tb_0 = halves[0][0]
            first_xt = []
            for k in range(KB):
                xtk = xp.tile([PB, tb_0], fp32r, tag=f"xt{k}_0", name=f"xt{k}_0")
                nc.sync.dma_start(
                    out=xtk[:], in_=xT[k * PB:(k + 1) * PB, t0_0:t0_0 + tb_0])
                first_xt.append(xtk)
                wgk = w1q0.tile([PB, QW], fp32r, name=f"wg{k}_0")
                nc.sync.dma_start(out=wgk[:], in_=Wg[k * PB:(k + 1) * PB, 0:QW])
                wg_s[k][0] = wgk
            for k in range(KB):
                wuk = w1q0.tile([PB, QW], fp32r, name=f"wu{k}_0")
                nc.sync.dma_start(out=wuk[:], in_=Wu[k * PB:(k + 1) * PB, 0:QW])
                wu_s[k][0] = wuk
            # Rest of half-0 x tiles, then remaining weight quarters.
            xt_half.append([first_xt] + [load_xt(si, t0, tb)
                                         for si, (t0, tb)
                                         in enumerate(halves[0][1:], start=1)])
            for q in range(1, NQ):
                for k in range(KB):
                    wgk = w1.tile([PB, QW], fp32r, name=f"wg{k}_{q}")
                    nc.sync.dma_start(
                        out=wgk[:], in_=Wg[k * PB:(k + 1) * PB, q * QW:(q + 1) * QW])
                    wg_s[k][q] = wgk
                for k in range(KB):
                    wuk = w1.tile([PB, QW], fp32r, name=f"wu{k}_{q}")
                    nc.sync.dma_start(
                        out=wuk[:], in_=Wu[k * PB:(k + 1) * PB, q * QW:(q + 1) * QW])
                    wu_s[k][q] = wuk
            # Preload the first Wd tiles during phase 1 (disjoint SBUF).
            wd_early = []
            for ib in range(N_EARLY):
                wdk = wde.tile([PB, H], fp32r, name=f"wde{ib}")
                nc.sync.dma_start(out=wdk[:], in_=Wd[ib * PB:(ib + 1) * PB, :])
                wd_early.append(wdk)

            for hi, half in enumerate(halves):
                if hi > 0:
                    xt_half.append([load_xt(si, t0, tb)
                                    for si, (t0, tb) in enumerate(half)])
                for q in range(NQ):
                    if hi == len(halves) - 1 and q == 1:
                        # q0 is dead everywhere: release its SBUF and use it
                        # to preload 8 more Wd tiles during the phase-1 tail.
                        w1q0_stack.close()
                        w2a = _stack.enter_context(
                            tc.tile_pool(name="w2a", bufs=1, side="right"))
                        for ib in range(N_EARLY, min(IB, N_EARLY + 8)):
                            wdk = w2a.tile([PB, H], fp32r, name=f"wda{ib}")
                            nc.sync.dma_start(
                                out=wdk[:], in_=Wd[ib * PB:(ib + 1) * PB, :])
                            wd_early.append(wdk)
                    for si, (t0, tb) in enumerate(half):
                        xt = xt_half[hi][si]
                        for ii in range(IPQ):
                            ib = q * IPQ + ii
                            qc = ii * PB
                            pg = ps1.tile([PB, tb], fp32, tag="pg", name="pg")
                            pu = ps1.tile([PB, tb], fp32, tag="pu", name="pu")
                            for k in range(KB):
                                nc.tensor.matmul(
                                    pg[:],
                                    wg_s[k][q][:, qc:qc + PB],
                                    xt[k][:],
                                    start=(k == 0), stop=(k == KB - 1))
                            for k in range(KB):
                                nc.tensor.matmul(
                                    pu[:],
                                    wu_s[k][q][:, qc:qc + PB],
                                    xt[k][:],
                                    start=(k == 0), stop=(k == KB - 1))
                            sg = ev1.tile([PB, tb], fp32, tag="sg", name="sg")
                            nc.scalar.activation(sg[:], pg[:], AF.Sigmoid)
                            sx = ev1.tile([PB, tb], fp32, tag="sx", name="sx")
                            nc.vector.tensor_mul(sx[:], sg[:], pg[:])
                            hh = ev1.tile([PB, tb], fp32r, tag="hh", name="hh")
                            nc.vector.tensor_mul(hh[:], sx[:], pu[:])
                            nc.gpsimd.dma_start(
                                out=hsp[ib * PB:(ib + 1) * PB, t0:t0 + tb],
                                in_=hh[:])

        # Phase 2: yT = (h @ Wd) * gate.  ib-outer: all 8 output blocks
        # accumulate in 8 PSUM banks so compute starts after wd0+ht0 land.
        with tc.tile_pool(name="w2", bufs=1) as w2, \
             tc.tile_pool(name="hl", bufs=3) as hl, \
             tc.tile_pool(name="ev2", bufs=8) as ev2, \
             tc.tile_pool(name="ps2", bufs=1, space="PSUM") as ps2:
            def load_ht(t0, tb, ib):
                htk = hl.tile([PB, tb], fp32r, tag=f"ht{ib}", name=f"ht{ib}")
                nc.sync.dma_start(
                    out=htk[:], in_=hsp[ib * PB:(ib + 1) * PB, t0:t0 + tb])
                return htk

            # Interleave remaining wd tiles with seg-0 h tiles in need-order.
            wd_s = list(wd_early)
            ht_next = []   # seg0 tiles
            for ib in range(IB):
                if ib >= len(wd_early):
                    wdk = w2.tile([PB, H], fp32r, name=f"wd{ib}")
                    nc.sync.dma_start(out=wdk[:], in_=Wd[ib * PB:(ib + 1) * PB, :])
                    wd_s.append(wdk)
                ht_next.append(load_ht(segs[0][0], segs[0][1], ib))
            gt = w2.tile([PB, C], fp32, name="gt")
            nc.sync.dma_start(out=gt[:], in_=gm[:])
            for si, (t0, tb) in enumerate(segs):
                ht = ht_next
                # queue the next segment's h tiles
                if si + 1 < len(segs):
                    nt0, ntb = segs[si + 1]
                    ht_next = [load_ht(nt0, ntb, ib) for ib in range(IB)]
                py = [ps2.tile([PB, tb], fp32, tag=f"py{hb}", name=f"py{hb}")
                      for hb in range(HB)]
                for ib in range(IB):
                    last = ib == IB - 1
                    for hb in range(HB):
                        nc.tensor.matmul(
                            py[hb][:],
                            wd_s[ib][:, hb * PB:(hb + 1) * PB],
                            ht[ib][:],
                            start=(ib == 0), stop=last)
                        if last:
                            # evict as soon as this output block finishes;
                            # the final segment flushes on the idle HWDGE
                            # queue (all loads are done by then).
                            yt = ev2.tile([PB, tb], fp32, tag="yt", name="yt")
                            nc.vector.tensor_mul(yt[:], py[hb][:],
                                                 gt[:, t0:t0 + tb])
                            eng = (nc.sync if si == len(segs) - 1
                                   else nc.gpsimd)
                            eng.dma_start(
                                out=yT[hb * PB:(hb + 1) * PB, t0:t0 + tb],
                                in_=yt[:])
    nc.compile()
    return nc


def _route(x, Wr, br):
    """Replicate the reference's fp32 router bit-compatibly on host."""
    logits = x @ Wr + br                       # fp32 GEMM
    order = np.argsort(-logits, axis=1, kind="stable")  # ties -> lowest index
    topk_idx = order[:, :TOPK]
    topk_vals = np.take_along_axis(logits, topk_idx, axis=1)
    g = 1.0 / (1.0 + np.exp(-topk_vals.astype(np.float32)))
    g = g / (np.sum(g, axis=-1, keepdims=True) + 1e-10)
    return topk_idx, g.astype(np.float32)


def kernel(x, Wr, br, Wg, Wu, Wd):
    global last_results
    from concourse.bass_utils import run_bass_kernel_spmd

    x = np.asarray(x, dtype=np.float32)
    Wr = np.asarray(Wr, dtype=np.float32)
    br = np.asarray(br, dtype=np.float32)
    Wg = np.asarray(Wg, dtype=np.float32)
    Wu = np.asarray(Wu, dtype=np.float32)
    Wd = np.asarray(Wd, dtype=np.float32)

    topk_idx, g = _route(x, Wr, br)

    # Per-expert token lists
    idx_lists = []
    gate_lists = []
    for e in range(E):
        mask = topk_idx == e                    # [T, K]
        tok = np.nonzero(mask.any(axis=1))[0]
        # gate value for expert e per selected token (slot 0 or slot 1)
        gsel = np.where(mask[tok, 0], g[tok, 0], g[tok, 1]).astype(np.float32)
        idx_lists.append(tok.astype(np.int64))
        gate_lists.append(gsel)

    counts = [len(ix) for ix in idx_lists]
    C = max(512, max(counts))

    key = C
    if key not in _compiled:
        _compiled[key] = _build(C)
    nc = _compiled[key]

    xTf = round_fp32r(np.ascontiguousarray(x.T))   # [H, T], pre-rounded
    in_maps = []
    for e in range(E):
        n = counts[e]
        xTe = np.zeros((H, C), dtype=np.float32)
        xTe[:, :n] = xTf[:, idx_lists[e]]
        gme = np.zeros((PB, C), dtype=np.float32)
        gme[:, :n] = gate_lists[e][None, :]
        in_maps.append({
            "xT": xTe,
            "gm": gme,
            "Wg": round_fp32r(Wg[e]),
            "Wu": round_fp32r(Wu[e]),
            "Wd": round_fp32r(Wd[e]),
        })

    trace = bool(int(os.environ.get("MOE_TRACE", "0")))
    trace_cores = (list(range(NCORES))
                   if os.environ.get("MOE_TRACE_ALL") else None)
    last_results = run_bass_kernel_spmd(
        nc, in_maps, core_ids=list(range(NCORES)), trace=trace,
        trace_cores=trace_cores)

    out = np.zeros((T, H), dtype=np.float32)
    for e in range(E):
        n = counts[e]
        yTe = last_results.results[e]["yT"]
        out[idx_lists[e]] += yTe[:, :n].T
    return out
